# revision 1
# baseline (speedup 1.0000x reference)
"""nn_ALIKED NMS-detection kernel for 8 TRN2 NeuronCores.

Device (Bass, SPMD x8): dense 5x5 max-pool map of the scores image — the
memory-bound bulk of the DKD pipeline. Sharding: data-parallel over batch,
each image's rows split across 2 cores (4 images x 2 half-images = 8 cores).
Host: shard/unshard + sparse tail (top-k select, 5x5 patch soft-argmax,
bilinear resample) on the 8192 keypoints/image.
"""
import sys

import numpy as np

sys.path.insert(0, "/opt/trn_rl_repo")

from concourse import bass, mybir  # noqa: E402
from concourse.bass_utils import run_bass_kernel_spmd  # noqa: E402
from concourse.tile import TileContext  # noqa: E402

B, H, W = 4, 1536, 1536
MXALU = None
RAD = 2
K = 5
TOP_K = 8192
TEMP = 0.1

# per-core shard: 768 output rows + 4 halo rows, width padded by 2 each side
SH_ROWS = H // 2 + 2 * RAD  # 772
SH_COLS = W + 2 * RAD  # 1540
OUT_ROWS = H // 2  # 768
N_BLK = OUT_ROWS // 128  # 6

f32 = mybir.dt.float32
MX = mybir.AluOpType.max
_nc_cache = None


STARTS = [124 * j for j in range(6)] + [SH_ROWS - 128]
NB = len(STARTS)
MXOP = None

def _build():
    nc = bass.Bass()
    x = nc.declare_dram_parameter("x", [SH_ROWS, SH_COLS], f32, isOutput=False)
    sh = nc.declare_dram_parameter("shifts", [128, 3 * 128], f32, isOutput=False)
    out = nc.declare_dram_parameter("out", [OUT_ROWS, W], f32, isOutput=True)
    ap = lambda h, off, pat: bass.AP(h if isinstance(h, bass.bass_types.TensorHandle) else h, off, pat) if False else bass.AP(h, off, pat)
    from contextlib import ExitStack
    es = ExitStack()
    with es:
        ta = es.enter_context(nc.sbuf_tensor("ta", [128, SH_COLS], f32))
        tb = es.enter_context(nc.sbuf_tensor("tb", [128, SH_COLS], f32))
        shs = es.enter_context(nc.sbuf_tensor("shs", [128, 3 * 128], f32))
        m2 = es.enter_context(nc.sbuf_tensor("m2", [128, SH_COLS - 1], f32))
        m4 = es.enter_context(nc.sbuf_tensor("m4", [128, SH_COLS - 3], f32))
        ca = es.enter_context(nc.sbuf_tensor("ca", [128, W], f32))
        cb = es.enter_context(nc.sbuf_tensor("cb", [128, W], f32))
        v2t = es.enter_context(nc.sbuf_tensor("v2", [128, W], f32))
        v4t = es.enter_context(nc.sbuf_tensor("v4", [128, W], f32))
        v5a = es.enter_context(nc.sbuf_tensor("v5a", [128, W], f32))
        v5b = es.enter_context(nc.sbuf_tensor("v5b", [128, W], f32))
        pa = es.enter_context(nc.psum_tensor("pa", [128, W], f32))
        pb = es.enter_context(nc.psum_tensor("pb", [128, W], f32))
        block = es.enter_context(nc.Block())
        dma_sem = es.enter_context(nc.semaphore("dma_sem"))
        sh_sem = es.enter_context(nc.semaphore("sh_sem"))
        g_sem = es.enter_context(nc.semaphore("g_sem"))
        c_sem = es.enter_context(nc.semaphore("c_sem"))
        pe_sem = es.enter_context(nc.semaphore("pe_sem"))
        v2_sem = es.enter_context(nc.semaphore("v2_sem"))
        v_sem = es.enter_context(nc.semaphore("v_sem"))
        st_sem = es.enter_context(nc.semaphore("st_sem"))
        ts = [ta, tb]
        cs = [ca, cb]
        v5s = [v5a, v5b]
        A2 = lambda h, off, n: bass.AP(h, off, [[h.shape[1] if hasattr(h,'shape') else 0, 128], [1, n]])

        def tile_ap(h, cols, off, n):
            return bass.AP(h, off, [[cols, 128], [1, n]])

        @block.sync
        def _(sync):
            sync.dma_start(out=shs[:, :], in_=sh[:, :]).then_inc(sh_sem, 16)
            sync.dma_start(out=ts[0][:, :], in_=x[STARTS[0] : STARTS[0] + 128, :]).then_inc(dma_sem, 16)
            sync.dma_start(out=ts[1][:, :], in_=x[STARTS[1] : STARTS[1] + 128, :]).then_inc(dma_sem, 16)
            for j in range(NB):
                sync.wait_ge(v_sem, j + 1)
                sync.dma_start(out=out[STARTS[j] : STARTS[j] + 124, :], in_=v5s[j % 2][0:124, :]).then_inc(st_sem, 16)
                if j + 2 < NB:
                    sync.wait_ge(c_sem, j + 1)
                    sync.dma_start(out=ts[j % 2][:, :], in_=x[STARTS[j + 2] : STARTS[j + 2] + 128, :]).then_inc(dma_sem, 16)
            sync.wait_ge(st_sem, 16 * NB)

        @block.tensor
        def _(te):
            te.wait_ge(sh_sem, 16)
            for j in range(NB):
                if j >= 1:
                    te.wait_ge(v_sem, j)
                te.wait_ge(c_sem, j + 1)
                c = cs[j % 2]
                for d_i, dst in ((0, pa), (2, pb)):
                    for ck in range(3):
                        mm = te.matmul(
                            bass.AP(dst, 512 * ck, [[W, 128], [1, 512]]),
                            bass.AP(shs, 128 * d_i, [[3 * 128, 128], [1, 128]]),
                            bass.AP(c, 512 * ck, [[W, 128], [1, 512]]),
                        )
                        if ck == 2:
                            mm.then_inc(pe_sem, 1)
                te.wait_ge(v2_sem, j + 1)
                for ck in range(3):
                    mm = te.matmul(
                        bass.AP(pa, 512 * ck, [[W, 128], [1, 512]]),
                        bass.AP(shs, 128 * 1, [[3 * 128, 128], [1, 128]]),
                        bass.AP(v2t, 512 * ck, [[W, 128], [1, 512]]),
                    )
                    if ck == 2:
                        mm.then_inc(pe_sem, 1)

        @block.vector
        def _(ve):
            def colmax(j):
                ve.wait_ge(dma_sem, 16 * (j + 1))
                t = ts[j % 2]
                c = cs[j % 2]
                ve.tensor_tensor(out=m2[:, :], in0=t[:, 0 : SH_COLS - 1], in1=t[:, 1:SH_COLS], op=MX)
                ve.tensor_tensor(out=m4[:, :], in0=m2[:, 0 : SH_COLS - 3], in1=m2[:, 2 : SH_COLS - 1], op=MX)
                ve.tensor_tensor(out=c[:, :], in0=m4[:, 0:W], in1=t[:, 4:SH_COLS], op=MX).then_inc(c_sem, 1)

            colmax(0)
            for j in range(NB):
                if j + 1 < NB:
                    colmax(j + 1)  # fills the DVE bubble while PE shifts block j
                c = cs[j % 2]
                ve.wait_ge(pe_sem, 3 * j + 1)
                ve.tensor_tensor(out=v2t[:, :], in0=c[:, :], in1=pa[:, :], op=MX).then_inc(v2_sem, 1)
                ve.wait_ge(pe_sem, 3 * j + 3)
                ve.tensor_tensor(out=v4t[:, :], in0=v2t[:, :], in1=pa[:, :], op=MX)
                if j >= 2:
                    ve.wait_ge(st_sem, 16 * (j - 1))
                ve.tensor_tensor(out=v5s[j % 2][:, :], in0=v4t[:, :], in1=pb[:, :], op=MX).then_inc(v_sem, 1)
    return nc



def _device_maxpool(s):
    """s: (B, H, W) f32 -> (B, H, W) 5x5 window max, computed on 8 NeuronCores."""
    global _nc_cache
    if _nc_cache is None:
        _nc_cache = _build()
    sp = np.pad(s, ((0, 0), (RAD, RAD), (RAD, RAD)))  # zero-pad: values >= 0 so
    # max with 0-pad == max with -inf pad here (window always contains center >= 0)
    shifts = np.zeros((128, 3, 128), np.float32)
    for i, d in enumerate((1, 2, 4)):
        for pp in range(128 - d):
            shifts[pp + d, i, pp] = 1.0
    shifts = np.ascontiguousarray(shifts.reshape(128, 384))
    in_maps = []
    for b in range(B):
        in_maps.append({"x": np.ascontiguousarray(sp[b, 0:SH_ROWS, :]), "shifts": shifts})
        in_maps.append({"x": np.ascontiguousarray(sp[b, H // 2 : H // 2 + SH_ROWS, :]), "shifts": shifts})
    res = run_bass_kernel_spmd(_nc_cache, in_maps, list(range(8)))
    mx = np.empty((B, H, W), np.float32)
    for b in range(B):
        mx[b, 0 : H // 2] = res.results[2 * b]["out"]
        mx[b, H // 2 :] = res.results[2 * b + 1]["out"]
    return mx


def kernel(scores_map: np.ndarray) -> np.ndarray:
    s = np.asarray(scores_map, dtype=np.float32).reshape(B, H, W)

    mx = _device_maxpool(s)

    # --- host tail: NMS mask, border zero, exact top-k, soft-argmax refine ---
    nms = np.where(s == mx, s, np.float32(0.0))
    nms[:, :RAD, :] = 0.0
    nms[:, -RAD:, :] = 0.0
    nms[:, :, :RAD] = 0.0
    nms[:, :, -RAD:] = 0.0

    flat = nms.reshape(B, H * W)
    # stable argsort of -v == sort by (value desc, idx asc): matches lax.top_k
    idx = np.empty((B, TOP_K), np.int64)
    for b in range(B):
        idx[b] = np.argsort(-flat[b], kind="stable")[:TOP_K]
    ky = (idx // W).astype(np.int64)
    kx = (idx % W).astype(np.int64)

    sp = np.pad(s, ((0, 0), (RAD, RAD), (RAD, RAD)))
    offs = np.arange(K)
    dy, dx = np.meshgrid(offs, offs, indexing="ij")
    dy = dy.reshape(-1)
    dx = dx.reshape(-1)
    bidx = np.arange(B)[:, None, None]
    patch = sp[bidx, ky[:, :, None] + dy[None, None], kx[:, :, None] + dx[None, None]]
    patch = patch.astype(np.float32)  # (B, M, 25)

    max_v = patch.max(axis=-1, keepdims=True)
    x_exp = np.exp((patch - max_v) / np.float32(TEMP), dtype=np.float32)
    denom = x_exp.sum(axis=-1, keepdims=True, dtype=np.float32)
    grid = (np.stack([dx, dy], axis=-1).astype(np.float32) - RAD)  # (25, 2)
    xy_res = (x_exp @ grid) / denom  # (B, M, 2)

    dist2 = (((grid[None, None] - xy_res[:, :, None, :]) / RAD) ** 2).sum(axis=-1)
    dispersity = (x_exp * dist2).sum(axis=-1) / denom[..., 0]

    kp = np.stack([kx, ky], axis=-1).astype(np.float32) + xy_res
    wh = np.asarray([W - 1, H - 1], np.float32)
    kpn = kp / wh * np.float32(2.0) - np.float32(1.0)

    px = (kpn[..., 0] + 1.0) * 0.5 * (W - 1)
    py = (kpn[..., 1] + 1.0) * 0.5 * (H - 1)
    x0 = np.clip(np.floor(px).astype(np.int64), 0, W - 2)
    y0 = np.clip(np.floor(py).astype(np.int64), 0, H - 2)
    wx = (px - x0).astype(np.float32)
    wy = (py - y0).astype(np.float32)
    b2 = np.arange(B)[:, None]
    v00 = s[b2, y0, x0]
    v01 = s[b2, y0, x0 + 1]
    v10 = s[b2, y0 + 1, x0]
    v11 = s[b2, y0 + 1, x0 + 1]
    kptscore = ((1 - wx) * (1 - wy) * v00 + wx * (1 - wy) * v01
                + (1 - wx) * wy * v10 + wx * wy * v11)

    out = np.concatenate(
        [kpn, kptscore[..., None], dispersity[..., None]], axis=-1
    ).astype(np.float32)
    return out



# revision 2
# speedup vs baseline: 4.5583x; 4.5583x over previous
"""nn_ALIKED NMS-detection kernel for 8 TRN2 NeuronCores.

Device (Bass, SPMD x8): dense 5x5-window NMS *screen* over a monotone 4-bit
quantization of the scores map — the memory-bound bulk of the DKD pipeline.
Each core handles half an image (4 images x 2 half-images = 8 cores) and
returns a bit-packed candidate mask (pixels that tie with their 5x5 window
max in 4-bit space). Because the quantization is monotone, the candidate set
is a strict superset of the exact f32 NMS maxima for ANY input.

Host: exact f32 verification of the top candidates (gathers 5x5 patches for
~the top 16K candidates per image and keeps true f32 local maxima), then
top-k select, 5x5 soft-argmax refinement, dispersity and bilinear score
resampling on the 8192 keypoints/image. A full-precision host fallback
covers degenerate inputs (fewer than 8192 positive-score maxima), so
correctness never depends on the input distribution.

Transfer budget per call (the dominant cost through the axon tunnel):
input 8 x 772x770 u8 = 4.76MB, output (+donated zeros) 2 x 1.18MB, vs the
naive f32 maxpool round trip of ~115MB.
"""
import sys

import numpy as np

sys.path.insert(0, "/opt/trn_rl_repo")

from concourse import bass, mybir  # noqa: E402
from concourse.bass_utils import run_bass_kernel_spmd  # noqa: E402

B, H, W = 4, 1536, 1536
RAD = 2
K = 5
TOP_K = 8192
TEMP = 0.1

HALF = H // 2  # 768 rows per core
SH_ROWS = HALF + 2 * RAD  # 772 input rows per core (with halo)
PAD_COLS = W + 2 * RAD  # 1540 padded columns
PACK_COLS = PAD_COLS // 2  # 770 bytes per row (2 4-bit pixels per byte)
PK_COLS = W // 8  # 192 bytes of packed output mask per row
NB = HALF // 128  # 6 blocks of 128 output rows

u8 = mybir.dt.uint8
MX = mybir.AluOpType.max
EQ = mybir.AluOpType.is_equal
AND = mybir.AluOpType.bitwise_and
SHR = mybir.AluOpType.logical_shift_right
SHL = mybir.AluOpType.logical_shift_left
OR = mybir.AluOpType.bitwise_or

_nc_cache = None


def _build():
    """5x5 NMS screen on 4-bit scores, bit-packed mask output.

    Input x: (772, 770) u8, two 4-bit pixels per byte (lo nibble = even
    padded col, hi nibble = odd padded col), zero padding baked in.
    Output out: (768, 192) u8, bit k of byte c8 = candidate flag for
    output pixel column 8*c8+k (little bit order).
    """
    nc = bass.Bass()
    x = nc.declare_dram_parameter("x", [SH_ROWS, PACK_COLS], u8, isOutput=False)
    out = nc.declare_dram_parameter("out", [HALF, PK_COLS], u8, isOutput=True)
    from contextlib import ExitStack

    es = ExitStack()
    with es:
        # double-buffered input tiles: 5 row-shifted copies per block
        t = [
            [es.enter_context(nc.sbuf_tensor(f"t{bb}_{d}", [128, PACK_COLS], u8)) for d in range(5)]
            for bb in range(2)
        ]
        lo = [es.enter_context(nc.sbuf_tensor(f"lo{d}", [128, PACK_COLS], u8)) for d in range(5)]
        hi = [es.enter_context(nc.sbuf_tensor(f"hi{d}", [128, PACK_COLS], u8)) for d in range(5)]
        rl1 = es.enter_context(nc.sbuf_tensor("rl1", [128, PACK_COLS], u8))
        rl2 = es.enter_context(nc.sbuf_tensor("rl2", [128, PACK_COLS], u8))
        rl3 = es.enter_context(nc.sbuf_tensor("rl3", [128, PACK_COLS], u8))
        rlo = es.enter_context(nc.sbuf_tensor("rlo", [128, PACK_COLS], u8))
        rh1 = es.enter_context(nc.sbuf_tensor("rh1", [128, PACK_COLS], u8))
        rh2 = es.enter_context(nc.sbuf_tensor("rh2", [128, PACK_COLS], u8))
        rh3 = es.enter_context(nc.sbuf_tensor("rh3", [128, PACK_COLS], u8))
        rhi = es.enter_context(nc.sbuf_tensor("rhi", [128, PACK_COLS], u8))
        ee2 = es.enter_context(nc.sbuf_tensor("ee2", [128, PACK_COLS], u8))
        ee3 = es.enter_context(nc.sbuf_tensor("ee3", [128, PACK_COLS], u8))
        oo2 = es.enter_context(nc.sbuf_tensor("oo2", [128, PACK_COLS], u8))
        oo3 = es.enter_context(nc.sbuf_tensor("oo3", [128, PACK_COLS], u8))
        rev = es.enter_context(nc.sbuf_tensor("rev", [128, W // 2], u8))
        rod = es.enter_context(nc.sbuf_tensor("rod", [128, W // 2], u8))
        me = es.enter_context(nc.sbuf_tensor("me", [128, W // 2], u8))
        mo = es.enter_context(nc.sbuf_tensor("mo", [128, W // 2], u8))
        tt = [es.enter_context(nc.sbuf_tensor(f"tt{i}", [128, PK_COLS], u8)) for i in range(2)]
        acc = [es.enter_context(nc.sbuf_tensor(f"acc{i}", [128, PK_COLS], u8)) for i in range(2)]
        pk = [es.enter_context(nc.sbuf_tensor(f"pk{i}", [128, PK_COLS], u8)) for i in range(2)]
        block = es.enter_context(nc.Block())
        dsem = es.enter_context(nc.semaphore("dsem"))
        vsem = es.enter_context(nc.semaphore("vsem"))
        ssem = es.enter_context(nc.semaphore("ssem"))

        def load_block(sync, j):
            r0 = 128 * j
            for d in range(5):
                sync.dma_start(out=t[j % 2][d][:, :], in_=x[r0 + d : r0 + d + 128, :]).then_inc(dsem, 16)

        @block.sync
        def _(sync):
            load_block(sync, 0)
            load_block(sync, 1)
            for j in range(NB):
                sync.wait_ge(vsem, j + 1)
                sync.dma_start(out=out[128 * j : 128 * (j + 1), :], in_=pk[j % 2][:, :]).then_inc(ssem, 16)
                if j + 2 < NB:
                    load_block(sync, j + 2)
            sync.wait_ge(ssem, 16 * NB)

        @block.vector
        def _(ve):
            E = W // 2  # 768
            for j in range(NB):
                ve.wait_ge(dsem, 80 * (j + 1))
                tj = t[j % 2]
                for d in range(5):
                    ve.tensor_scalar(out=lo[d][:, :], in0=tj[d][:, :], scalar1=15, scalar2=None, op0=AND)
                    ve.tensor_scalar(out=hi[d][:, :], in0=tj[d][:, :], scalar1=4, scalar2=None, op0=SHR)
                # 5-row max of even (lo) / odd (hi) nibble planes
                ve.tensor_tensor(out=rl1[:, :], in0=lo[0][:, :], in1=lo[1][:, :], op=MX)
                ve.tensor_tensor(out=rl2[:, :], in0=lo[2][:, :], in1=lo[3][:, :], op=MX)
                ve.tensor_tensor(out=rl3[:, :], in0=rl1[:, :], in1=rl2[:, :], op=MX)
                ve.tensor_tensor(out=rlo[:, :], in0=rl3[:, :], in1=lo[4][:, :], op=MX)
                ve.tensor_tensor(out=rh1[:, :], in0=hi[0][:, :], in1=hi[1][:, :], op=MX)
                ve.tensor_tensor(out=rh2[:, :], in0=hi[2][:, :], in1=hi[3][:, :], op=MX)
                ve.tensor_tensor(out=rh3[:, :], in0=rh1[:, :], in1=rh2[:, :], op=MX)
                ve.tensor_tensor(out=rhi[:, :], in0=rh3[:, :], in1=hi[4][:, :], op=MX)
                # 5-col max over interleaved even/odd planes:
                # E[i]=rlo[i] is padded col 2i, O[i]=rhi[i] is padded col 2i+1.
                # out col 2i   -> max(E[i..i+2], O[i..i+1]) = max(EE3[i], OO2[i])
                # out col 2i+1 -> max(O[i..i+2], E[i+1..i+2]) = max(OO3[i], EE2[i+1])
                ve.tensor_tensor(out=ee2[:, 0:769], in0=rlo[:, 0:769], in1=rlo[:, 1:770], op=MX)
                ve.tensor_tensor(out=ee3[:, 0:768], in0=ee2[:, 0:768], in1=rlo[:, 2:770], op=MX)
                ve.tensor_tensor(out=oo2[:, 0:769], in0=rhi[:, 0:769], in1=rhi[:, 1:770], op=MX)
                ve.tensor_tensor(out=oo3[:, 0:768], in0=oo2[:, 0:768], in1=rhi[:, 2:770], op=MX)
                ve.tensor_tensor(out=rev[:, :], in0=ee3[:, 0:768], in1=oo2[:, 0:768], op=MX)
                ve.tensor_tensor(out=rod[:, :], in0=oo3[:, 0:768], in1=ee2[:, 1:769], op=MX)
                # candidate flags: center nibble equals its 5x5 window max
                ve.tensor_tensor(out=me[:, :], in0=lo[2][:, 1:769], in1=rev[:, :], op=EQ)
                ve.tensor_tensor(out=mo[:, :], in0=hi[2][:, 1:769], in1=rod[:, :], op=EQ)
                # bit-pack: bit 2m <- me[:, m::4], bit 2m+1 <- mo[:, m::4]
                if j >= 2:
                    ve.wait_ge(ssem, 16 * (j - 1))
                ve.tensor_copy(out=acc[0][:, :], in_=bass.AP(me, 0, [[E, 128], [4, PK_COLS]]))
                step = 0
                for m in range(4):
                    for par, src in ((0, me), (1, mo)):
                        bit = 2 * m + par
                        if bit == 0:
                            continue
                        ve.tensor_scalar(
                            out=tt[step % 2][:, :],
                            in0=bass.AP(src, m, [[E, 128], [4, PK_COLS]]),
                            scalar1=bit,
                            scalar2=None,
                            op0=SHL,
                        )
                        dst = pk[j % 2] if bit == 7 else acc[(step + 1) % 2]
                        ve.tensor_tensor(
                            out=dst[:, :], in0=acc[step % 2][:, :], in1=tt[step % 2][:, :], op=OR
                        )
                        step += 1
                ve.drain().then_inc(vsem, 1)

    return nc


def _in_maps(s):
    """s: (B, H, W) f32 -> list of 8 per-core input dicts (4-bit packed)."""
    q = np.clip(s * 16.0, 0.0, 15.0).astype(np.uint8)  # monotone 4-bit quantization
    qp = np.zeros((B, H + 2 * RAD, PAD_COLS), np.uint8)
    qp[:, RAD : RAD + H, RAD : RAD + W] = q
    xp = qp[:, :, 0::2] | (qp[:, :, 1::2] << 4)  # (B, 1540, 770)
    maps = []
    for b in range(B):
        for h in range(2):
            maps.append({"x": np.ascontiguousarray(xp[b, h * HALF : h * HALF + SH_ROWS, :])})
    return maps


def _device_screen(s):
    """s: (B, H, W) f32 -> (B, H, W) u8 candidate mask, computed on 8 cores."""
    global _nc_cache
    if _nc_cache is None:
        _nc_cache = _build()
    res = run_bass_kernel_spmd(_nc_cache, _in_maps(s), list(range(8)))
    flg = np.empty((B, H, W), np.uint8)
    for b in range(B):
        for h in range(2):
            flg[b, h * HALF : (h + 1) * HALF] = np.unpackbits(
                res.results[2 * b + h]["out"], axis=1, bitorder="little"
            )
    return flg


_offs = np.arange(K)
_dy, _dx = np.meshgrid(_offs, _offs, indexing="ij")
_dy = _dy.reshape(-1)  # (25,) row offsets 0..4
_dx = _dx.reshape(-1)  # (25,) col offsets 0..4


def _host_full_select(sb):
    """Exact reference-equivalent selection on one image (fallback path)."""
    pp = np.full((H + 2 * RAD, W + 2 * RAD), -np.inf, np.float32)
    pp[RAD : RAD + H, RAD : RAD + W] = sb
    m = pp
    c1 = np.maximum(m[:, 0 : W + 3], m[:, 1 : W + 4])
    c2 = np.maximum(c1[:, 0 : W + 1], c1[:, 2 : W + 3])
    cm = np.maximum(c2[:, 0:W], m[:, 4 : W + 4])  # (H+4, W) col-window-5 max
    r1 = np.maximum(cm[0 : H + 3], cm[1 : H + 4])
    r2 = np.maximum(r1[0 : H + 1], r1[2 : H + 3])
    mx = np.maximum(r2[0:H], cm[4 : H + 4])  # (H, W) 5x5 max
    nms = np.where(sb == mx, sb, np.float32(0.0))
    nms[:RAD] = 0.0
    nms[-RAD:] = 0.0
    nms[:, :RAD] = 0.0
    nms[:, -RAD:] = 0.0
    idx = np.argsort(-nms.reshape(-1), kind="stable")[:TOP_K]
    return (idx // W).astype(np.int64), (idx % W).astype(np.int64)


def kernel(scores_map: np.ndarray) -> np.ndarray:
    s = np.asarray(scores_map, dtype=np.float32).reshape(B, H, W)

    flg = _device_screen(s)

    # zero the border flags (reference zeroes a RAD-wide border after NMS)
    flg[:, :RAD] = 0
    flg[:, -RAD:] = 0
    flg[:, :, :RAD] = 0
    flg[:, :, -RAD:] = 0

    ky_all = np.empty((B, TOP_K), np.int64)
    kx_all = np.empty((B, TOP_K), np.int64)
    patch_all = np.empty((B, TOP_K, K * K), np.float32)

    for b in range(B):
        sb = s[b]
        sp = np.pad(sb, RAD)  # zero pad, only ever read for border pixels
        ys, xs = np.nonzero(flg[b])  # row-major: ascending flat index
        v = sb[ys, xs]

        sel_ky = sel_kx = sel_patch = None
        ncand = len(v)
        N0 = 16384
        while True:
            if ncand == 0:
                break
            if ncand > N0:
                top = np.argpartition(-v, N0 - 1)[:N0]
                vmin = v[top].min()
                sel = np.nonzero(v >= vmin)[0]  # all boundary ties included
            else:
                sel = np.arange(ncand)
            # exact reference order: value desc, flat index asc (stable)
            order = sel[np.argsort(-v[sel], kind="stable")]
            oy = ys[order]
            ox = xs[order]
            patch = sp[oy[:, None] + _dy[None], ox[:, None] + _dx[None]]  # (n, 25)
            true = v[order] == patch.max(axis=1)  # exact f32 local-max test
            rows = np.flatnonzero(true)
            if len(rows) >= TOP_K:
                rows = rows[:TOP_K]
                if v[order[rows[-1]]] <= 0.0:
                    sel_ky = None  # zero-score tail: defer to exact fallback
                    break
                sel_ky = oy[rows]
                sel_kx = ox[rows]
                sel_patch = patch[rows].astype(np.float32)
                break
            if ncand <= N0:
                break  # not enough true maxima among candidates: fallback
            N0 *= 4

        if sel_ky is None:
            sel_ky, sel_kx = _host_full_select(sb)
            sel_patch = sp[sel_ky[:, None] + _dy[None], sel_kx[:, None] + _dx[None]].astype(
                np.float32
            )
        ky_all[b] = sel_ky
        kx_all[b] = sel_kx
        patch_all[b] = sel_patch

    # --- soft-argmax refinement, dispersity, bilinear resample (as reference) ---
    ky = ky_all
    kx = kx_all
    patch = patch_all

    max_v = patch.max(axis=-1, keepdims=True)
    x_exp = np.exp((patch - max_v) / np.float32(TEMP), dtype=np.float32)
    denom = x_exp.sum(axis=-1, keepdims=True, dtype=np.float32)
    grid = np.stack([_dx, _dy], axis=-1).astype(np.float32) - RAD  # (25, 2)
    xy_res = (x_exp @ grid) / denom  # (B, M, 2)

    dist2 = (((grid[None, None] - xy_res[:, :, None, :]) / RAD) ** 2).sum(axis=-1)
    dispersity = (x_exp * dist2).sum(axis=-1) / denom[..., 0]

    kp = np.stack([kx, ky], axis=-1).astype(np.float32) + xy_res
    wh = np.asarray([W - 1, H - 1], np.float32)
    kpn = kp / wh * np.float32(2.0) - np.float32(1.0)

    px = (kpn[..., 0] + 1.0) * 0.5 * (W - 1)
    py = (kpn[..., 1] + 1.0) * 0.5 * (H - 1)
    x0 = np.clip(np.floor(px).astype(np.int64), 0, W - 2)
    y0 = np.clip(np.floor(py).astype(np.int64), 0, H - 2)
    wx = (px - x0).astype(np.float32)
    wy = (py - y0).astype(np.float32)
    b2 = np.arange(B)[:, None]
    v00 = s[b2, y0, x0]
    v01 = s[b2, y0, x0 + 1]
    v10 = s[b2, y0 + 1, x0]
    v11 = s[b2, y0 + 1, x0 + 1]
    kptscore = ((1 - wx) * (1 - wy) * v00 + wx * (1 - wy) * v01
                + (1 - wx) * wy * v10 + wx * wy * v11)

    out = np.concatenate(
        [kpn, kptscore[..., None], dispersity[..., None]], axis=-1
    ).astype(np.float32)
    return out


# revision 3
# speedup vs baseline: 6.0283x; 1.3225x over previous
"""nn_ALIKED NMS-detection kernel for 8 TRN2 NeuronCores.

Device (Bass, SPMD x8): dense 5x5-window NMS *screen* over a monotone
non-uniform 2-bit quantization of the scores map — the memory-bound bulk of
the DKD pipeline. Each core handles half an image (4 images x 2 half-images
= 8 cores) and returns a bit-packed candidate mask (pixels that tie with
their 5x5 window max in 2-bit space). Because the quantization is monotone,
the candidate set is a strict superset of the exact f32 NMS maxima for ANY
input; bin edges (48, 60, 63)/64 concentrate resolution near 1.0 where the
top-k cutoff for a dense scores map lives.

Host: exact f32 verification of the top candidates (gathers 5x5 patches and
keeps true f32 local maxima, in exact (value desc, index asc) reference
order), then 5x5 soft-argmax refinement, dispersity and bilinear score
resampling on the 8192 keypoints/image. Adaptive guards (top-bin fast path
-> all candidates -> full-precision host fallback) make correctness
independent of the input distribution.

Transfer budget per call (the dominant cost through the axon tunnel):
input 8 x 772x385 u8 = 2.38MB, output (+donated zeros) 2 x 1.18MB, vs the
naive f32 maxpool round trip of ~115MB.
"""
import sys
from concurrent.futures import ThreadPoolExecutor

import numpy as np

sys.path.insert(0, "/opt/trn_rl_repo")

from concourse import bass, mybir  # noqa: E402
from concourse.bass_utils import run_bass_kernel_spmd  # noqa: E402

B, H, W = 4, 1536, 1536
RAD = 2
K = 5
TOP_K = 8192
TEMP = 0.1

HALF = H // 2  # 768 rows per core
SH_ROWS = HALF + 2 * RAD  # 772 input rows per core (with halo)
PAD_COLS = W + 2 * RAD  # 1540 padded columns
PACK_COLS = PAD_COLS // 4  # 385 bytes per row (4 2-bit pixels per byte)
PK_COLS = W // 8  # 192 bytes of packed output mask per row
NB = HALF // 128  # 6 blocks of 128 output rows
NQ = W // 4  # 384 output columns per residue class

# non-uniform 2-bit bin edges, in units of 1/64 (monotone for any input)
QEDGES = (48, 60, 63)
T_TOP = np.float32(QEDGES[2] / 64.0)  # value floor of the top bin

u8 = mybir.dt.uint8
MX = mybir.AluOpType.max
EQ = mybir.AluOpType.is_equal
AND = mybir.AluOpType.bitwise_and
SHR = mybir.AluOpType.logical_shift_right
SHL = mybir.AluOpType.logical_shift_left
OR = mybir.AluOpType.bitwise_or

_nc_cache = None


def _build():
    """5x5 NMS screen on 2-bit scores, bit-packed mask output.

    Input x: (772, 385) u8, four 2-bit pixels per byte (bits 2p:2p+1 = padded
    col 4i+p of byte i), zero padding baked in. Output out: (768, 192) u8,
    bit k of byte c8 = candidate flag for output pixel column 8*c8+k.
    """
    nc = bass.Bass()
    x = nc.declare_dram_parameter("x", [SH_ROWS, PACK_COLS], u8, isOutput=False)
    out = nc.declare_dram_parameter("out", [HALF, PK_COLS], u8, isOutput=True)
    from contextlib import ExitStack

    es = ExitStack()
    with es:
        # double-buffered input tiles: 5 row-shifted copies per block
        t = [
            [es.enter_context(nc.sbuf_tensor(f"t{bb}_{d}", [128, PACK_COLS], u8)) for d in range(5)]
            for bb in range(2)
        ]
        # 2-bit planes per tile: plane p holds padded cols == p (mod 4)
        pl = [
            [es.enter_context(nc.sbuf_tensor(f"pl{d}_{p}", [128, PACK_COLS], u8)) for p in range(4)]
            for d in range(5)
        ]
        w1 = es.enter_context(nc.sbuf_tensor("w1", [128, PACK_COLS], u8))
        w2 = es.enter_context(nc.sbuf_tensor("w2", [128, PACK_COLS], u8))
        w3 = es.enter_context(nc.sbuf_tensor("w3", [128, PACK_COLS], u8))
        A = [es.enter_context(nc.sbuf_tensor(f"A{p}", [128, PACK_COLS], u8)) for p in range(4)]
        p01 = es.enter_context(nc.sbuf_tensor("p01", [128, PACK_COLS], u8))
        p23 = es.enter_context(nc.sbuf_tensor("p23", [128, PACK_COLS], u8))
        qq = es.enter_context(nc.sbuf_tensor("qq", [128, PACK_COLS], u8))
        m123 = es.enter_context(nc.sbuf_tensor("m123", [128, PACK_COLS], u8))
        t012 = es.enter_context(nc.sbuf_tensor("t012", [128, PACK_COLS], u8))
        r = [es.enter_context(nc.sbuf_tensor(f"r{i}", [128, NQ], u8)) for i in range(4)]
        m = [es.enter_context(nc.sbuf_tensor(f"m{i}", [128, NQ], u8)) for i in range(4)]
        tt = [es.enter_context(nc.sbuf_tensor(f"tt{i}", [128, PK_COLS], u8)) for i in range(2)]
        acc = [es.enter_context(nc.sbuf_tensor(f"acc{i}", [128, PK_COLS], u8)) for i in range(2)]
        pk = [es.enter_context(nc.sbuf_tensor(f"pk{i}", [128, PK_COLS], u8)) for i in range(2)]
        block = es.enter_context(nc.Block())
        dsem = es.enter_context(nc.semaphore("dsem"))
        vsem = es.enter_context(nc.semaphore("vsem"))
        ssem = es.enter_context(nc.semaphore("ssem"))

        def load_block(sync, j):
            r0 = 128 * j
            for d in range(5):
                sync.dma_start(out=t[j % 2][d][:, :], in_=x[r0 + d : r0 + d + 128, :]).then_inc(dsem, 16)

        @block.sync
        def _(sync):
            load_block(sync, 0)
            load_block(sync, 1)
            for j in range(NB):
                sync.wait_ge(vsem, j + 1)
                sync.dma_start(out=out[128 * j : 128 * (j + 1), :], in_=pk[j % 2][:, :]).then_inc(ssem, 16)
                if j + 2 < NB:
                    load_block(sync, j + 2)
            sync.wait_ge(ssem, 16 * NB)

        @block.vector
        def _(ve):
            for j in range(NB):
                ve.wait_ge(dsem, 80 * (j + 1))
                tj = t[j % 2]
                for d in range(5):
                    ve.tensor_scalar(out=pl[d][0][:, :], in0=tj[d][:, :], scalar1=3, scalar2=None, op0=AND)
                    ve.tensor_scalar(out=pl[d][1][:, :], in0=tj[d][:, :], scalar1=2, scalar2=3, op0=SHR, op1=AND)
                    ve.tensor_scalar(out=pl[d][2][:, :], in0=tj[d][:, :], scalar1=4, scalar2=3, op0=SHR, op1=AND)
                    ve.tensor_scalar(out=pl[d][3][:, :], in0=tj[d][:, :], scalar1=6, scalar2=None, op0=SHR)
                # 5-row max per residue plane
                for p in range(4):
                    ve.tensor_tensor(out=w1[:, :], in0=pl[0][p][:, :], in1=pl[1][p][:, :], op=MX)
                    ve.tensor_tensor(out=w2[:, :], in0=pl[2][p][:, :], in1=pl[3][p][:, :], op=MX)
                    ve.tensor_tensor(out=w3[:, :], in0=w1[:, :], in1=w2[:, :], op=MX)
                    ve.tensor_tensor(out=A[p][:, :], in0=w3[:, :], in1=pl[4][p][:, :], op=MX)
                # cross-plane combos
                ve.tensor_tensor(out=p01[:, :], in0=A[0][:, :], in1=A[1][:, :], op=MX)
                ve.tensor_tensor(out=p23[:, :], in0=A[2][:, :], in1=A[3][:, :], op=MX)
                ve.tensor_tensor(out=qq[:, :], in0=p01[:, :], in1=p23[:, :], op=MX)
                ve.tensor_tensor(out=m123[:, :], in0=p23[:, :], in1=A[1][:, :], op=MX)
                ve.tensor_tensor(out=t012[:, :], in0=p01[:, :], in1=A[2][:, :], op=MX)
                # 5-col window max, out col 4i+r covers padded cols 4i+r..4i+r+4
                ve.tensor_tensor(out=r[0][:, :], in0=qq[:, 0:NQ], in1=A[0][:, 1 : NQ + 1], op=MX)
                ve.tensor_tensor(out=r[1][:, :], in0=m123[:, 0:NQ], in1=p01[:, 1 : NQ + 1], op=MX)
                ve.tensor_tensor(out=r[2][:, :], in0=p23[:, 0:NQ], in1=t012[:, 1 : NQ + 1], op=MX)
                ve.tensor_tensor(out=r[3][:, :], in0=A[3][:, 0:NQ], in1=qq[:, 1 : NQ + 1], op=MX)
                # candidate flags: center 2-bit value equals its 5x5 window max
                # center of out col 4i+r is padded col 4i+r+2 (from tile d=2)
                ve.tensor_tensor(out=m[0][:, :], in0=pl[2][2][:, 0:NQ], in1=r[0][:, :], op=EQ)
                ve.tensor_tensor(out=m[1][:, :], in0=pl[2][3][:, 0:NQ], in1=r[1][:, :], op=EQ)
                ve.tensor_tensor(out=m[2][:, :], in0=pl[2][0][:, 1 : NQ + 1], in1=r[2][:, :], op=EQ)
                ve.tensor_tensor(out=m[3][:, :], in0=pl[2][1][:, 1 : NQ + 1], in1=r[3][:, :], op=EQ)
                # bit-pack: bit k of byte c8 <- m[k%4][:, (k//4)::2] at index 2*c8
                if j >= 2:
                    ve.wait_ge(ssem, 16 * (j - 1))
                ve.tensor_copy(out=acc[0][:, :], in_=bass.AP(m[0], 0, [[NQ, 128], [2, PK_COLS]]))
                for bit in range(1, 8):
                    step = bit - 1
                    ve.tensor_scalar(
                        out=tt[step % 2][:, :],
                        in0=bass.AP(m[bit % 4], bit // 4, [[NQ, 128], [2, PK_COLS]]),
                        scalar1=bit,
                        scalar2=None,
                        op0=SHL,
                    )
                    dst = pk[j % 2] if bit == 7 else acc[(step + 1) % 2]
                    ve.tensor_tensor(
                        out=dst[:, :], in0=acc[step % 2][:, :], in1=tt[step % 2][:, :], op=OR
                    )
                ve.drain().then_inc(vsem, 1)

    return nc


_LUT = np.zeros(64, np.uint8)
_LUT[QEDGES[0] :] = 1
_LUT[QEDGES[1] :] = 2
_LUT[QEDGES[2] :] = 3


def _shard_pack(s, b, h):
    """Quantize + 2-bit-pack one core's shard of the scores map."""
    r0 = h * HALF
    q2p = np.zeros((SH_ROWS, PAD_COLS), np.uint8)
    lo = max(0, r0 - RAD)
    hi = min(H, r0 + HALF + RAD)
    q6 = np.clip(s[b, lo:hi] * np.float32(64.0), 0.0, 63.0).astype(np.uint8)
    q2p[lo - (r0 - RAD) : hi - (r0 - RAD), RAD : RAD + W] = _LUT[q6]
    return (
        q2p[:, 0::4]
        | (q2p[:, 1::4] << 2)
        | (q2p[:, 2::4] << 4)
        | (q2p[:, 3::4] << 6)
    )


def _in_maps(s):
    """s: (B, H, W) f32 -> list of 8 per-core input dicts (2-bit packed)."""
    with ThreadPoolExecutor(8) as ex:
        xs = list(ex.map(lambda c: _shard_pack(s, c // 2, c % 2), range(2 * B)))
    return [{"x": xp} for xp in xs]


def _device_screen(s):
    """s: (B, H, W) f32 -> (B, H, W) u8 candidate mask, computed on 8 cores."""
    global _nc_cache
    if _nc_cache is None:
        _nc_cache = _build()
    res = run_bass_kernel_spmd(_nc_cache, _in_maps(s), list(range(8)))
    flg = np.empty((B, H, W), np.uint8)
    for b in range(B):
        for h in range(2):
            flg[b, h * HALF : (h + 1) * HALF] = np.unpackbits(
                res.results[2 * b + h]["out"], axis=1, bitorder="little"
            )
    return flg


_offs = np.arange(K)
_dy, _dx = np.meshgrid(_offs, _offs, indexing="ij")
_dy = _dy.reshape(-1)  # (25,) row offsets 0..4
_dx = _dx.reshape(-1)  # (25,) col offsets 0..4


def _select_from(ys, xs, v, sp, allow_zero_tail=False):
    """Pick the top-8192 exact f32 local maxima among candidate pixels, in
    exact reference order (value desc, flat index asc). Returns
    (ky, kx, patches) or None if the candidate set can't supply 8192."""
    ncand = len(v)
    N0 = 16384
    while True:
        if ncand == 0:
            return None
        if ncand > N0:
            top = np.argpartition(-v, N0 - 1)[:N0]
            vmin = v[top].min()
            sel = np.nonzero(v >= vmin)[0]  # all boundary ties included
        else:
            sel = np.arange(ncand)
        order = sel[np.argsort(-v[sel], kind="stable")]
        oy = ys[order]
        ox = xs[order]
        patch = sp[oy[:, None] + _dy[None], ox[:, None] + _dx[None]]  # (n, 25)
        true = v[order] == patch.max(axis=1)  # exact f32 local-max test
        rows = np.flatnonzero(true)
        if len(rows) >= TOP_K:
            rows = rows[:TOP_K]
            if not allow_zero_tail and v[order[rows[-1]]] <= 0.0:
                return None  # zero-score tail: defer to exact fallback
            return oy[rows], ox[rows], patch[rows].astype(np.float32)
        if ncand <= N0:
            return None
        N0 *= 4


def _host_full_select(sb):
    """Exact reference-equivalent selection on one image (fallback path)."""
    pp = np.full((H + 2 * RAD, W + 2 * RAD), -np.inf, np.float32)
    pp[RAD : RAD + H, RAD : RAD + W] = sb
    m = pp
    c1 = np.maximum(m[:, 0 : W + 3], m[:, 1 : W + 4])
    c2 = np.maximum(c1[:, 0 : W + 1], c1[:, 2 : W + 3])
    cm = np.maximum(c2[:, 0:W], m[:, 4 : W + 4])  # (H+4, W) col-window-5 max
    r1 = np.maximum(cm[0 : H + 3], cm[1 : H + 4])
    r2 = np.maximum(r1[0 : H + 1], r1[2 : H + 3])
    mx = np.maximum(r2[0:H], cm[4 : H + 4])  # (H, W) 5x5 max
    nms = np.where(sb == mx, sb, np.float32(0.0))
    nms[:RAD] = 0.0
    nms[-RAD:] = 0.0
    nms[:, :RAD] = 0.0
    nms[:, -RAD:] = 0.0
    idx = np.argsort(-nms.reshape(-1), kind="stable")[:TOP_K]
    return (idx // W).astype(np.int64), (idx % W).astype(np.int64)


def _image_tail(sb, flgb):
    """One image: candidates -> exact top-k selection -> (ky, kx, patches)."""
    sp = np.pad(sb, RAD)  # zero pad, only ever read for border pixels

    # fast path: candidates in the top quantization bin
    topmask = sb >= T_TOP
    np.logical_and(topmask, flgb.view(bool), out=topmask)
    ys, xs = np.nonzero(topmask)
    res = None
    if len(ys):
        res = _select_from(ys, xs, sb[ys, xs], sp)
    if res is None:
        # all device candidates (exact superset of true maxima)
        ys, xs = np.nonzero(flgb)
        if len(ys):
            res = _select_from(ys, xs, sb[ys, xs], sp)
    if res is None:
        # exact full-precision fallback (degenerate inputs)
        ky, kx = _host_full_select(sb)
        patch = sp[ky[:, None] + _dy[None], kx[:, None] + _dx[None]].astype(np.float32)
        res = (ky, kx, patch)
    return res


def kernel(scores_map: np.ndarray) -> np.ndarray:
    s = np.asarray(scores_map, dtype=np.float32).reshape(B, H, W)

    flg = _device_screen(s)

    # zero the border flags (reference zeroes a RAD-wide border after NMS)
    flg[:, :RAD] = 0
    flg[:, -RAD:] = 0
    flg[:, :, :RAD] = 0
    flg[:, :, -RAD:] = 0

    with ThreadPoolExecutor(B) as ex:
        tails = list(ex.map(lambda b: _image_tail(s[b], flg[b]), range(B)))

    ky = np.stack([t[0] for t in tails]).astype(np.int64)
    kx = np.stack([t[1] for t in tails]).astype(np.int64)
    patch = np.stack([t[2] for t in tails])  # (B, M, 25) f32

    # --- soft-argmax refinement, dispersity, bilinear resample (as reference) ---
    max_v = patch.max(axis=-1, keepdims=True)
    x_exp = np.exp((patch - max_v) / np.float32(TEMP), dtype=np.float32)
    denom = x_exp.sum(axis=-1, keepdims=True, dtype=np.float32)
    grid = np.stack([_dx, _dy], axis=-1).astype(np.float32) - RAD  # (25, 2)
    xy_res = (x_exp @ grid) / denom  # (B, M, 2)

    dist2 = (((grid[None, None] - xy_res[:, :, None, :]) / RAD) ** 2).sum(axis=-1)
    dispersity = (x_exp * dist2).sum(axis=-1) / denom[..., 0]

    kp = np.stack([kx, ky], axis=-1).astype(np.float32) + xy_res
    wh = np.asarray([W - 1, H - 1], np.float32)
    kpn = kp / wh * np.float32(2.0) - np.float32(1.0)

    px = (kpn[..., 0] + 1.0) * 0.5 * (W - 1)
    py = (kpn[..., 1] + 1.0) * 0.5 * (H - 1)
    x0 = np.clip(np.floor(px).astype(np.int64), 0, W - 2)
    y0 = np.clip(np.floor(py).astype(np.int64), 0, H - 2)
    wx = (px - x0).astype(np.float32)
    wy = (py - y0).astype(np.float32)
    b2 = np.arange(B)[:, None]
    v00 = s[b2, y0, x0]
    v01 = s[b2, y0, x0 + 1]
    v10 = s[b2, y0 + 1, x0]
    v11 = s[b2, y0 + 1, x0 + 1]
    kptscore = ((1 - wx) * (1 - wy) * v00 + wx * (1 - wy) * v01
                + (1 - wx) * wy * v10 + wx * wy * v11)

    out = np.concatenate(
        [kpn, kptscore[..., None], dispersity[..., None]], axis=-1
    ).astype(np.float32)
    return out


# revision 6
# speedup vs baseline: 10.4290x; 1.7300x over previous
"""nn_ALIKED NMS-detection kernel for 8 TRN2 NeuronCores.

Device (Bass, SPMD x8): dense 5x5-window NMS *screen* over a monotone
non-uniform 2-bit quantization of the scores map — the memory-bound bulk of
the DKD pipeline. Each core handles half an image (4 images x 2 half-images
= 8 cores) and returns a bit-packed candidate mask (pixels that tie with
their 5x5 window max in 2-bit space). Because the quantization is monotone,
the candidate set is a strict superset of the exact f32 NMS maxima for ANY
input; bin edges (48, 60, 63)/64 concentrate resolution near 1.0 where the
top-k cutoff for a dense scores map lives.

Host: exact f32 verification of the top candidates (gathers 5x5 patches and
keeps true f32 local maxima, in exact (value desc, index asc) reference
order), then 5x5 soft-argmax refinement, dispersity and bilinear score
resampling on the 8192 keypoints/image. Adaptive guards (top-bin fast path
-> all candidates -> full-precision host fallback) make correctness
independent of the input distribution.

Transfer budget per call (the dominant cost through the axon tunnel):
input 8 x 772x385 u8 = 2.38MB, output (+donated zeros) 2 x 1.18MB, vs the
naive f32 maxpool round trip of ~115MB.
"""
import sys
from concurrent.futures import ThreadPoolExecutor

import numpy as np

sys.path.insert(0, "/opt/trn_rl_repo")

import jax  # noqa: E402

try:
    # Persistent executable cache: run_bass_kernel_spmd re-jits its closure
    # every call, so without this each call re-runs the client-side BIR
    # compile pipeline (~350ms). With it, repeat calls deserialize from disk.
    jax.config.update("jax_compilation_cache_dir", "/tmp/jax_pcache")
    jax.config.update("jax_persistent_cache_min_entry_size_bytes", -1)
    jax.config.update("jax_persistent_cache_min_compile_time_secs", 0.0)
except Exception:  # noqa: BLE001
    pass

from concourse import bass, mybir  # noqa: E402
from concourse.bass_utils import run_bass_kernel_spmd  # noqa: E402

B, H, W = 4, 1536, 1536
RAD = 2
K = 5
TOP_K = 8192
TEMP = 0.1

HALF = H // 2  # 768 rows per core
SH_ROWS = HALF + 2 * RAD  # 772 input rows per core (with halo)
PAD_COLS = W + 2 * RAD  # 1540 padded columns
PACK_COLS = PAD_COLS // 4  # 385 bytes per row (4 2-bit pixels per byte)
PK_COLS = W // 8  # 192 bytes of packed output mask per row
NB = HALF // 128  # 6 blocks of 128 output rows
NQ = W // 4  # 384 output columns per residue class

# non-uniform 2-bit bin edges, in units of 1/64 (monotone for any input)
QEDGES = (48, 60, 63)
T_TOP = np.float32(QEDGES[2] / 64.0)  # value floor of the top bin

u8 = mybir.dt.uint8
MX = mybir.AluOpType.max
EQ = mybir.AluOpType.is_equal
AND = mybir.AluOpType.bitwise_and
SHR = mybir.AluOpType.logical_shift_right
SHL = mybir.AluOpType.logical_shift_left
OR = mybir.AluOpType.bitwise_or

_nc_cache = None


def _build():
    """5x5 NMS screen on 2-bit scores, bit-packed mask output.

    Input x: (772, 385) u8, four 2-bit pixels per byte (bits 2p:2p+1 = padded
    col 4i+p of byte i), zero padding baked in. Output out: (768, 192) u8,
    bit k of byte c8 = candidate flag for output pixel column 8*c8+k.
    """
    nc = bass.Bass()
    x = nc.declare_dram_parameter("x", [SH_ROWS, PACK_COLS], u8, isOutput=False)
    out = nc.declare_dram_parameter("out", [HALF, PK_COLS], u8, isOutput=True)
    from contextlib import ExitStack

    es = ExitStack()
    with es:
        # double-buffered input tiles: 5 row-shifted copies per block
        t = [
            [es.enter_context(nc.sbuf_tensor(f"t{bb}_{d}", [128, PACK_COLS], u8)) for d in range(5)]
            for bb in range(2)
        ]
        # 2-bit planes per tile: plane p holds padded cols == p (mod 4)
        pl = [
            [es.enter_context(nc.sbuf_tensor(f"pl{d}_{p}", [128, PACK_COLS], u8)) for p in range(4)]
            for d in range(5)
        ]
        w1 = es.enter_context(nc.sbuf_tensor("w1", [128, PACK_COLS], u8))
        w2 = es.enter_context(nc.sbuf_tensor("w2", [128, PACK_COLS], u8))
        w3 = es.enter_context(nc.sbuf_tensor("w3", [128, PACK_COLS], u8))
        A = [es.enter_context(nc.sbuf_tensor(f"A{p}", [128, PACK_COLS], u8)) for p in range(4)]
        p01 = es.enter_context(nc.sbuf_tensor("p01", [128, PACK_COLS], u8))
        p23 = es.enter_context(nc.sbuf_tensor("p23", [128, PACK_COLS], u8))
        qq = es.enter_context(nc.sbuf_tensor("qq", [128, PACK_COLS], u8))
        m123 = es.enter_context(nc.sbuf_tensor("m123", [128, PACK_COLS], u8))
        t012 = es.enter_context(nc.sbuf_tensor("t012", [128, PACK_COLS], u8))
        r = [es.enter_context(nc.sbuf_tensor(f"r{i}", [128, NQ], u8)) for i in range(4)]
        m = [es.enter_context(nc.sbuf_tensor(f"m{i}", [128, NQ], u8)) for i in range(4)]
        tt = [es.enter_context(nc.sbuf_tensor(f"tt{i}", [128, PK_COLS], u8)) for i in range(2)]
        acc = [es.enter_context(nc.sbuf_tensor(f"acc{i}", [128, PK_COLS], u8)) for i in range(2)]
        pk = [es.enter_context(nc.sbuf_tensor(f"pk{i}", [128, PK_COLS], u8)) for i in range(2)]
        block = es.enter_context(nc.Block())
        dsem = es.enter_context(nc.semaphore("dsem"))
        vsem = es.enter_context(nc.semaphore("vsem"))
        ssem = es.enter_context(nc.semaphore("ssem"))

        def load_block(sync, j):
            r0 = 128 * j
            for d in range(5):
                sync.dma_start(out=t[j % 2][d][:, :], in_=x[r0 + d : r0 + d + 128, :]).then_inc(dsem, 16)

        @block.sync
        def _(sync):
            load_block(sync, 0)
            load_block(sync, 1)
            for j in range(NB):
                sync.wait_ge(vsem, j + 1)
                sync.dma_start(out=out[128 * j : 128 * (j + 1), :], in_=pk[j % 2][:, :]).then_inc(ssem, 16)
                if j + 2 < NB:
                    load_block(sync, j + 2)
            sync.wait_ge(ssem, 16 * NB)

        @block.vector
        def _(ve):
            for j in range(NB):
                ve.wait_ge(dsem, 80 * (j + 1))
                tj = t[j % 2]
                for d in range(5):
                    ve.tensor_scalar(out=pl[d][0][:, :], in0=tj[d][:, :], scalar1=3, scalar2=None, op0=AND)
                    ve.tensor_scalar(out=pl[d][1][:, :], in0=tj[d][:, :], scalar1=2, scalar2=3, op0=SHR, op1=AND)
                    ve.tensor_scalar(out=pl[d][2][:, :], in0=tj[d][:, :], scalar1=4, scalar2=3, op0=SHR, op1=AND)
                    ve.tensor_scalar(out=pl[d][3][:, :], in0=tj[d][:, :], scalar1=6, scalar2=None, op0=SHR)
                # 5-row max per residue plane
                for p in range(4):
                    ve.tensor_tensor(out=w1[:, :], in0=pl[0][p][:, :], in1=pl[1][p][:, :], op=MX)
                    ve.tensor_tensor(out=w2[:, :], in0=pl[2][p][:, :], in1=pl[3][p][:, :], op=MX)
                    ve.tensor_tensor(out=w3[:, :], in0=w1[:, :], in1=w2[:, :], op=MX)
                    ve.tensor_tensor(out=A[p][:, :], in0=w3[:, :], in1=pl[4][p][:, :], op=MX)
                # cross-plane combos
                ve.tensor_tensor(out=p01[:, :], in0=A[0][:, :], in1=A[1][:, :], op=MX)
                ve.tensor_tensor(out=p23[:, :], in0=A[2][:, :], in1=A[3][:, :], op=MX)
                ve.tensor_tensor(out=qq[:, :], in0=p01[:, :], in1=p23[:, :], op=MX)
                ve.tensor_tensor(out=m123[:, :], in0=p23[:, :], in1=A[1][:, :], op=MX)
                ve.tensor_tensor(out=t012[:, :], in0=p01[:, :], in1=A[2][:, :], op=MX)
                # 5-col window max, out col 4i+r covers padded cols 4i+r..4i+r+4
                ve.tensor_tensor(out=r[0][:, :], in0=qq[:, 0:NQ], in1=A[0][:, 1 : NQ + 1], op=MX)
                ve.tensor_tensor(out=r[1][:, :], in0=m123[:, 0:NQ], in1=p01[:, 1 : NQ + 1], op=MX)
                ve.tensor_tensor(out=r[2][:, :], in0=p23[:, 0:NQ], in1=t012[:, 1 : NQ + 1], op=MX)
                ve.tensor_tensor(out=r[3][:, :], in0=A[3][:, 0:NQ], in1=qq[:, 1 : NQ + 1], op=MX)
                # candidate flags: center 2-bit value equals its 5x5 window max
                # center of out col 4i+r is padded col 4i+r+2 (from tile d=2)
                ve.tensor_tensor(out=m[0][:, :], in0=pl[2][2][:, 0:NQ], in1=r[0][:, :], op=EQ)
                ve.tensor_tensor(out=m[1][:, :], in0=pl[2][3][:, 0:NQ], in1=r[1][:, :], op=EQ)
                ve.tensor_tensor(out=m[2][:, :], in0=pl[2][0][:, 1 : NQ + 1], in1=r[2][:, :], op=EQ)
                ve.tensor_tensor(out=m[3][:, :], in0=pl[2][1][:, 1 : NQ + 1], in1=r[3][:, :], op=EQ)
                # bit-pack: bit k of byte c8 <- m[k%4][:, (k//4)::2] at index 2*c8
                if j >= 2:
                    ve.wait_ge(ssem, 16 * (j - 1))
                ve.tensor_copy(out=acc[0][:, :], in_=bass.AP(m[0], 0, [[NQ, 128], [2, PK_COLS]]))
                for bit in range(1, 8):
                    step = bit - 1
                    ve.tensor_scalar(
                        out=tt[step % 2][:, :],
                        in0=bass.AP(m[bit % 4], bit // 4, [[NQ, 128], [2, PK_COLS]]),
                        scalar1=bit,
                        scalar2=None,
                        op0=SHL,
                    )
                    dst = pk[j % 2] if bit == 7 else acc[(step + 1) % 2]
                    ve.tensor_tensor(
                        out=dst[:, :], in0=acc[step % 2][:, :], in1=tt[step % 2][:, :], op=OR
                    )
                ve.drain().then_inc(vsem, 1)

    return nc


# bin-edge thresholds as int32 bit patterns: for s >= 0 the IEEE-754 bits
# are monotone in the value, and any s < 0 views as a negative int32, which
# lands below every edge -> bin 0. Monotone for all real inputs.
_I1, _I2, _I3 = (np.float32(e / 64.0).view(np.int32).item() for e in QEDGES)


def _shard_pack(s, b, h):
    """Quantize + 2-bit-pack one core's shard of the scores map."""
    r0 = h * HALF
    lo = max(0, r0 - RAD)
    hi = min(H, r0 + HALF + RAD)
    iv = s[b, lo:hi].view(np.int32)
    q2 = (iv >= _I1).view(np.uint8) + (iv >= _I2).view(np.uint8)
    q2 += (iv >= _I3).view(np.uint8)
    xp = np.zeros((SH_ROWS, PACK_COLS), np.uint8)
    d0 = lo - (r0 - RAD)
    d1 = hi - (r0 - RAD)
    # byte i of a padded row holds padded cols 4i..4i+3 = image cols 4i-2..4i+1
    core = q2[:, 2:1534:4] | (q2[:, 3:1535:4] << 2)
    core |= q2[:, 4:1536:4] << 4
    core |= q2[:, 5:1536:4] << 6
    xp[d0:d1, 1 : PACK_COLS - 1] = core
    xp[d0:d1, 0] = (q2[:, 0] << 4) | (q2[:, 1] << 6)
    xp[d0:d1, PACK_COLS - 1] = q2[:, W - 2] | (q2[:, W - 1] << 2)
    return xp


def _in_maps(s):
    """s: (B, H, W) f32 -> list of 8 per-core input dicts (2-bit packed)."""
    with ThreadPoolExecutor(8) as ex:
        xs = list(ex.map(lambda c: _shard_pack(s, c // 2, c % 2), range(2 * B)))
    return [{"x": xp} for xp in xs]


def _device_screen(s):
    """s: (B, H, W) f32 -> (B, H, W) u8 candidate mask, computed on 8 cores."""
    global _nc_cache
    if _nc_cache is None:
        _nc_cache = _build()
    res = run_bass_kernel_spmd(_nc_cache, _in_maps(s), list(range(8)))
    flg = np.empty((B, H, W), np.uint8)
    for b in range(B):
        for h in range(2):
            flg[b, h * HALF : (h + 1) * HALF] = np.unpackbits(
                res.results[2 * b + h]["out"], axis=1, bitorder="little"
            )
    return flg


_offs = np.arange(K)
_dy, _dx = np.meshgrid(_offs, _offs, indexing="ij")
_dy = _dy.reshape(-1)  # (25,) row offsets 0..4
_dx = _dx.reshape(-1)  # (25,) col offsets 0..4


def _select_from(ys, xs, v, sp, allow_zero_tail=False):
    """Pick the top-8192 exact f32 local maxima among candidate pixels, in
    exact reference order (value desc, flat index asc). Returns
    (ky, kx, patches) or None if the candidate set can't supply 8192."""
    ncand = len(v)
    N0 = 16384
    while True:
        if ncand == 0:
            return None
        if ncand > N0:
            top = np.argpartition(-v, N0 - 1)[:N0]
            vmin = v[top].min()
            sel = np.nonzero(v >= vmin)[0]  # all boundary ties included
        else:
            sel = np.arange(ncand)
        order = sel[np.argsort(-v[sel], kind="stable")]
        oy = ys[order]
        ox = xs[order]
        patch = sp[oy[:, None] + _dy[None], ox[:, None] + _dx[None]]  # (n, 25)
        true = v[order] == patch.max(axis=1)  # exact f32 local-max test
        rows = np.flatnonzero(true)
        if len(rows) >= TOP_K:
            rows = rows[:TOP_K]
            if not allow_zero_tail and v[order[rows[-1]]] <= 0.0:
                return None  # zero-score tail: defer to exact fallback
            return oy[rows], ox[rows], patch[rows].astype(np.float32)
        if ncand <= N0:
            return None
        N0 *= 4


def _host_full_select(sb):
    """Exact reference-equivalent selection on one image (fallback path)."""
    pp = np.full((H + 2 * RAD, W + 2 * RAD), -np.inf, np.float32)
    pp[RAD : RAD + H, RAD : RAD + W] = sb
    m = pp
    c1 = np.maximum(m[:, 0 : W + 3], m[:, 1 : W + 4])
    c2 = np.maximum(c1[:, 0 : W + 1], c1[:, 2 : W + 3])
    cm = np.maximum(c2[:, 0:W], m[:, 4 : W + 4])  # (H+4, W) col-window-5 max
    r1 = np.maximum(cm[0 : H + 3], cm[1 : H + 4])
    r2 = np.maximum(r1[0 : H + 1], r1[2 : H + 3])
    mx = np.maximum(r2[0:H], cm[4 : H + 4])  # (H, W) 5x5 max
    nms = np.where(sb == mx, sb, np.float32(0.0))
    nms[:RAD] = 0.0
    nms[-RAD:] = 0.0
    nms[:, :RAD] = 0.0
    nms[:, -RAD:] = 0.0
    idx = np.argsort(-nms.reshape(-1), kind="stable")[:TOP_K]
    return (idx // W).astype(np.int64), (idx % W).astype(np.int64)


_grid = np.stack([_dx, _dy], axis=-1).astype(np.float32) - RAD  # (25, 2)


def _image_tail(sb, flgb):
    """One image: candidates -> exact top-k selection -> soft-argmax refine ->
    (M, 4) output rows [x_norm, y_norm, score, dispersity]."""
    sp = np.pad(sb, RAD)  # zero pad, only ever read for border pixels

    # fast path: candidates in the top quantization bin
    topmask = sb >= T_TOP
    np.logical_and(topmask, flgb.view(bool), out=topmask)
    ys, xs = np.nonzero(topmask)
    res = None
    if len(ys):
        res = _select_from(ys, xs, sb[ys, xs], sp)
    if res is None:
        # all device candidates (exact superset of true maxima)
        ys, xs = np.nonzero(flgb)
        if len(ys):
            res = _select_from(ys, xs, sb[ys, xs], sp)
    if res is None:
        # exact full-precision fallback (degenerate inputs)
        ky, kx = _host_full_select(sb)
        patch = sp[ky[:, None] + _dy[None], kx[:, None] + _dx[None]].astype(np.float32)
        res = (ky, kx, patch)
    ky, kx, patch = res

    # --- soft-argmax refinement, dispersity, bilinear resample (as reference) ---
    max_v = patch.max(axis=-1, keepdims=True)
    x_exp = np.exp((patch - max_v) / np.float32(TEMP), dtype=np.float32)
    denom = x_exp.sum(axis=-1, keepdims=True, dtype=np.float32)
    xy_res = (x_exp @ _grid) / denom  # (M, 2)

    dist2 = (((_grid[None] - xy_res[:, None, :]) / RAD) ** 2).sum(axis=-1)  # (M, 25)
    dispersity = (x_exp * dist2).sum(axis=-1) / denom[..., 0]

    kp = np.stack([kx, ky], axis=-1).astype(np.float32) + xy_res
    wh = np.asarray([W - 1, H - 1], np.float32)
    kpn = kp / wh * np.float32(2.0) - np.float32(1.0)

    px = (kpn[..., 0] + 1.0) * 0.5 * (W - 1)
    py = (kpn[..., 1] + 1.0) * 0.5 * (H - 1)
    x0 = np.clip(np.floor(px).astype(np.int64), 0, W - 2)
    y0 = np.clip(np.floor(py).astype(np.int64), 0, H - 2)
    wx = (px - x0).astype(np.float32)
    wy = (py - y0).astype(np.float32)
    v00 = sb[y0, x0]
    v01 = sb[y0, x0 + 1]
    v10 = sb[y0 + 1, x0]
    v11 = sb[y0 + 1, x0 + 1]
    kptscore = ((1 - wx) * (1 - wy) * v00 + wx * (1 - wy) * v01
                + (1 - wx) * wy * v10 + wx * wy * v11)

    return np.concatenate(
        [kpn, kptscore[:, None], dispersity[:, None]], axis=-1
    ).astype(np.float32)


def kernel(scores_map: np.ndarray) -> np.ndarray:
    s = np.asarray(scores_map, dtype=np.float32).reshape(B, H, W)

    flg = _device_screen(s)

    # zero the border flags (reference zeroes a RAD-wide border after NMS)
    flg[:, :RAD] = 0
    flg[:, -RAD:] = 0
    flg[:, :, :RAD] = 0
    flg[:, :, -RAD:] = 0

    with ThreadPoolExecutor(B) as ex:
        tails = list(ex.map(lambda b: _image_tail(s[b], flg[b]), range(B)))

    return np.stack(tails)
